# revision 4
# baseline (speedup 1.0000x reference)
"""Trainium2 Bass kernel for nn_LowRankRedistributor (8-core SPMD).

Math (reference):
    Lp    = L @ W @ W.T - L                     # [B, V] low-rank mint
    scale = max|L| / max(max|Lp|, EPS)
    out   = lerp(scale * Lp, L, ALPHA) = 0.6*scale*Lp + 0.4*L

Distribution: vocab-sharded tensor parallel over 8 cores (V=128000 ->
16000/core).  All on-device tensors are vocab-on-partitions
("transposed") so that no on-device transposes are needed:

  host passes per core (bf16):
    lt4 = (0.4*L).T shard    [16000, 256]   (scaled by ALPHA=0.4 so the
                                             final lerp needs no extra pass)
    w   = W shard            [16000, 64]
    wt  = W.T shard          [64, 16000]

  device (per core):
    pass1:  P' = sum_t w_t.T @ lt4_t  = 0.4*P.T (partial)   [64, 256] psum
            AllGather partials -> P4 = 0.4 * P.T (full)     [64, 256]
            maxL' = max|lt4|  (per-chunk abs-max reduces, DVE)
    pass2a: per vocab tile, PE computes Lp' in PSUM directly:
              psum  = wt_t.T @ P4  (+ accumulate)  (-I).T @ lt4_t
                    = 0.4*(M.T - L.T) = 0.4*Lp.T
            DVE:    maxLp' group-wise abs-max reduce (apply_absolute_value)
            ACT:    copy psum -> Lp' bf16 in SBUF
    scale:  partition_all_reduce + AllGather of [maxL', maxLp'] ->
            s15 = 1.5 * maxL' / max(maxLp', 0.4*EPS)  (= 1.5*scale)
    pass2b: fused (scalar_tensor_tensor): outT = s15*Lp' + lt4
            (= 0.6*scale*Lp.T + 0.4*L.T), DMA out as bf16.

  host:  gather shards, transpose back, cast to f32.

The 0.4/1.5/2.5 factors cancel exactly in the scale ratio, so the result
matches the reference up to bf16 precision (~0.3-0.5% rel err).
"""

import sys

for _p in ("/opt/trn_rl_repo",):
    if _p not in sys.path:
        sys.path.append(_p)

import numpy as np
import ml_dtypes

BATCH, VOCAB, RANK = 256, 128000, 64
N_CORES = 8
V_SHARD = VOCAB // N_CORES  # 16000
ALPHA = 0.4
EPS = float(np.finfo(np.float32).eps)

BF16 = ml_dtypes.bfloat16

_NC_CACHE = {}


def build_nc(v_shard=V_SHARD, batch=BATCH, rank=RANK, n_cores=N_CORES,
             n_chunks=5, grp=5):
    """Build + compile the SPMD Bass graph (per-core shard shapes)."""
    from concourse import bacc, tile, mybir, bass_isa

    F32 = mybir.dt.float32
    BF = mybir.dt.bfloat16
    AOP = mybir.AluOpType

    nt = v_shard // 128          # vocab tiles per core
    assert nt % n_chunks == 0
    tpc = nt // n_chunks         # tiles per DMA chunk
    assert tpc % grp == 0
    gpc = tpc // grp             # psum groups per chunk

    nc = bacc.Bacc("TRN2", target_bir_lowering=False, debug=False,
                   num_devices=n_cores)

    lt4 = nc.dram_tensor("lt4", [v_shard, batch], BF, kind="ExternalInput")
    w = nc.dram_tensor("w", [v_shard, rank], BF, kind="ExternalInput")
    wt = nc.dram_tensor("wt", [rank, v_shard], BF, kind="ExternalInput")
    outT = nc.dram_tensor("outT", [v_shard, batch], BF, kind="ExternalOutput")

    rg = [list(range(n_cores))]

    with tile.TileContext(nc) as tc:
        with (
            tc.tile_pool(name="persist", bufs=1) as persist,
            tc.tile_pool(name="small", bufs=1) as small,
            tc.tile_pool(name="oscr", bufs=4) as oscr_pool,
            tc.tile_pool(name="psum_p", bufs=1, space="PSUM") as psum_p,
            tc.tile_pool(name="psum_m", bufs=2, space="PSUM") as psum_m,
            tc.tile_pool(name="dram", bufs=1, space="DRAM") as dram,
        ):
            # ---- persistent SBUF ----
            lt4_sb = [persist.tile([128, tpc, batch], BF, name=f"lt4sb{c}")
                      for c in range(n_chunks)]
            lp_sb = [persist.tile([128, tpc, batch], BF, name=f"lpsb{c}")
                     for c in range(n_chunks)]
            wt_sb = persist.tile([rank, v_shard], BF, name="wtsb")
            neg_i = persist.tile([128, 128], BF, name="negi")

            maxl_cols = small.tile([128, n_chunks], F32, name="maxlcols")
            maxm_cols = small.tile([128, n_chunks * gpc], F32, name="maxmcols")

            # -I constant (for the PSUM subtraction matmuls)
            nc.gpsimd.memset(neg_i[:], 0.0)
            nc.gpsimd.affine_select(
                out=neg_i[:], in_=neg_i[:],
                compare_op=AOP.not_equal, fill=-1.0, base=0,
                pattern=[[-1, 128]], channel_multiplier=1)

            # ---- phase A: DMA in, pass1 matmuls, maxL reduces ----
            nc.sync.dma_start(wt_sb[:], wt.ap())

            psP = psum_p.tile([rank, batch], F32, name="psP")
            for c in range(n_chunks):
                rows = slice(c * tpc * 128, (c + 1) * tpc * 128)
                w_c = oscr_pool.tile([128, tpc, rank], BF, name="wc", tag="wc",
                                     bufs=2)
                nc.sync.dma_start(
                    w_c[:],
                    w.ap()[rows, :].rearrange("(n p) r -> p n r", p=128))
                nc.sync.dma_start(
                    lt4_sb[c][:],
                    lt4.ap()[rows, :].rearrange("(n p) b -> p n b", p=128))
                for i in range(tpc):
                    t = c * tpc + i
                    nc.tensor.matmul(
                        psP[:], w_c[:, i, :], lt4_sb[c][:, i, :],
                        start=(t == 0), stop=(t == nt - 1))
                nc.vector.tensor_reduce(
                    maxl_cols[:, c:c + 1],
                    lt4_sb[c][:].rearrange("p n b -> p (n b)"),
                    axis=mybir.AxisListType.X, op=AOP.max,
                    apply_absolute_value=True)

            # ---- phase B: AllGather partial P', sum ranks ----
            p4loc = small.tile([rank, batch], F32, name="p4loc")
            nc.vector.tensor_copy(p4loc[:], psP[:])
            ag1_in = dram.tile([rank * batch], F32, name="ag1in")
            ag1_out = dram.tile([n_cores * rank * batch], F32, name="ag1out")
            nc.sync.dma_start(
                ag1_in[:].rearrange("(p b) -> p b", p=rank), p4loc[:])
            nc.gpsimd.collective_compute(
                "AllGather", AOP.bypass, replica_groups=rg,
                ins=[ag1_in[:].opt()], outs=[ag1_out[:].opt()])
            p4all = small.tile([rank, n_cores, batch], F32, name="p4all")
            nc.sync.dma_start(
                p4all[:],
                ag1_out[:].rearrange("(r p b) -> p r b", r=n_cores, p=rank))
            p4f = small.tile([rank, batch], F32, name="p4f")
            nc.vector.tensor_reduce(
                p4f[:], p4all[:].rearrange("p r b -> p b r"),
                axis=mybir.AxisListType.X, op=AOP.add)
            p4bf = small.tile([rank, batch], BF, name="p4bf")
            nc.vector.tensor_copy(p4bf[:], p4f[:])

            # ---- phase C: PE computes Lp' = 0.4*(M.T - L.T) in PSUM;
            #      DVE abs-max reduces; ACT copies psum -> bf16 SBUF ----
            for c in range(n_chunks):
                for g in range(gpc):
                    gi = c * gpc + g
                    psM = psum_m.tile([128, grp * batch], F32, name="psM")
                    for i in range(grp):
                        ti = g * grp + i
                        t = (c * tpc + ti) * 128
                        sl = slice(i * batch, (i + 1) * batch)
                        nc.tensor.matmul(
                            psM[:, sl], wt_sb[:, t:t + 128], p4bf[:],
                            start=True, stop=False)
                        nc.tensor.matmul(
                            psM[:, sl], neg_i[:], lt4_sb[c][:, ti, :],
                            start=False, stop=True)
                    cols = slice(g * grp * batch, (g + 1) * grp * batch)
                    nc.vector.tensor_reduce(
                        maxm_cols[:, gi:gi + 1], psM[:],
                        axis=mybir.AxisListType.X, op=AOP.max,
                        apply_absolute_value=True)
                    nc.scalar.activation(
                        lp_sb[c][:].rearrange("p n b -> p (n b)")[:, cols],
                        psM[:], mybir.ActivationFunctionType.Copy)

            # ---- phase D: global scale ----
            maxs = small.tile([128, 2], F32, name="maxs")
            nc.vector.tensor_reduce(
                maxs[:, 0:1], maxl_cols[:], axis=mybir.AxisListType.X,
                op=AOP.max)
            nc.vector.tensor_reduce(
                maxs[:, 1:2], maxm_cols[:], axis=mybir.AxisListType.X,
                op=AOP.max)
            pmax = small.tile([128, 2], F32, name="pmax")
            nc.gpsimd.partition_all_reduce(
                pmax[:], maxs[:], channels=128,
                reduce_op=bass_isa.ReduceOp.max)
            ag2_in = dram.tile([2], F32, name="ag2in")
            ag2_out = dram.tile([n_cores * 2], F32, name="ag2out")
            nc.sync.dma_start(
                ag2_in[:].rearrange("(p b) -> p b", p=1), pmax[0:1, :])
            nc.gpsimd.collective_compute(
                "AllGather", AOP.bypass, replica_groups=rg,
                ins=[ag2_in[:].opt()], outs=[ag2_out[:].opt()])
            gmax = small.tile([1, n_cores * 2], F32, name="gmax")
            nc.sync.dma_start(
                gmax[:], ag2_out[:].rearrange("(p b) -> p b", p=1))
            gl = small.tile([1, 1], F32, name="gl")
            gm = small.tile([1, 1], F32, name="gm")
            gv = gmax[:].rearrange("p (r k) -> p r k", k=2)
            nc.vector.tensor_reduce(
                gl[:], gv[:, :, 0], axis=mybir.AxisListType.X, op=AOP.max)
            nc.vector.tensor_reduce(
                gm[:], gv[:, :, 1], axis=mybir.AxisListType.X, op=AOP.max)
            # s15 = 1.5 * gl / max(gm, 0.4*EPS)
            gmc = small.tile([1, 1], F32, name="gmc")
            nc.vector.tensor_scalar_max(gmc[:], gm[:], ALPHA * EPS)
            rec = small.tile([1, 1], F32, name="rec")
            nc.vector.reciprocal(rec[:], gmc[:])
            s1 = small.tile([1, 1], F32, name="s1")
            nc.vector.tensor_scalar(s1[:], gl[:], rec[:, 0:1], 1.5,
                                    op0=AOP.mult, op1=AOP.mult)
            s15 = small.tile([128, 1], F32, name="s15")
            nc.gpsimd.partition_broadcast(s15[:], s1[:])

            # ---- phase E: fused lerp + DMA out ----
            for c in range(n_chunks):
                for g in range(gpc):
                    cols = slice(g * grp * batch, (g + 1) * grp * batch)
                    osc = oscr_pool.tile([128, grp * batch], BF, name="osc")
                    nc.vector.scalar_tensor_tensor(
                        out=osc[:],
                        in0=lp_sb[c][:].rearrange("p n b -> p (n b)")[:, cols],
                        scalar=s15[:, 0:1],
                        in1=lt4_sb[c][:].rearrange("p n b -> p (n b)")[:, cols],
                        op0=AOP.mult, op1=AOP.add)
                    r0 = (c * tpc + g * grp) * 128
                    rows = slice(r0, r0 + grp * 128)
                    nc.sync.dma_start(
                        outT.ap()[rows, :].rearrange("(n p) b -> p n b", p=128),
                        osc[:].rearrange("p (n b) -> p n b", n=grp))
    nc.compile()
    return nc


def _get_nc():
    key = (V_SHARD, BATCH, RANK, N_CORES)
    if key not in _NC_CACHE:
        _NC_CACHE[key] = build_nc()
    return _NC_CACHE[key]


def kernel(**inputs) -> np.ndarray:
    from concourse import bass_utils

    logits = np.asarray(inputs["logits"], dtype=np.float32)
    W = np.asarray(inputs["W"], dtype=np.float32)
    assert logits.shape == (BATCH, VOCAB) and W.shape == (VOCAB, RANK)

    # host-side shard prep (bf16, transposed layouts)
    lt4_full = np.ascontiguousarray(
        (logits.T * np.float32(ALPHA)).astype(BF16))        # [V, B]
    w_full = W.astype(BF16)                                 # [V, R]
    wt_full = np.ascontiguousarray(W.T.astype(BF16))        # [R, V]

    in_maps = []
    for c in range(N_CORES):
        rows = slice(c * V_SHARD, (c + 1) * V_SHARD)
        in_maps.append({
            "lt4": lt4_full[rows],
            "w": w_full[rows],
            "wt": np.ascontiguousarray(wt_full[:, rows]),
        })

    nc = _get_nc()
    res = bass_utils.run_bass_kernel_spmd(
        nc, in_maps, core_ids=list(range(N_CORES)))

    outT = np.concatenate([res.results[c]["outT"] for c in range(N_CORES)],
                          axis=0)                           # [V, B] bf16
    return outT.T.astype(np.float32)


if __name__ == "__main__":
    rng = np.random.default_rng(0)
    L = rng.standard_normal((BATCH, VOCAB), dtype=np.float32)
    W = rng.standard_normal((VOCAB, RANK), dtype=np.float32)
    out = kernel(token_ids=np.zeros((BATCH, 1), np.int32), logits=L, W=W)
    Lp = (L @ W) @ W.T - L
    scale = np.abs(L).max() / max(np.abs(Lp).max(), EPS)
    ref = 0.6 * scale * Lp + 0.4 * L
    err = np.linalg.norm(out - ref) / np.linalg.norm(ref)
    print("rel err:", err)
